# revision 9
# baseline (speedup 1.0000x reference)
"""DeepFM (nn_DeepFM_77558519431939) Trainium2 Bass kernel — v4.

Strategy (8 NeuronCores, SPMD, no collectives):
  - Replicate the embedding table on every core; data-parallel the batch
    (16384 samples -> 2048 per core).
  - Host-side prep builds an augmented bf16 table [S, 12]: 10 embedding
    dims, w_first value in col 10, zero pad in col 11.
  - Gather: 128 single-offset indirect DMAs per core (the HW SWDGE ucode
    only supports the [128 offsets, one row each] form; batched multi-
    offset instructions generate corrupted descriptors).  Gather
    instructions can be spread across DGE channels (Pool SWDGE and,
    if supported, the SP/Act HWDGE rings) to cut the ~1.07us/instruction
    serialization.
  - Whole datapath is bf16 (PSUM accumulation fp32): PE matmuls and
    transposes at 1 cycle/row; numpy-verified end-to-end error ~3e-3
    vs the 2e-2 gate.
  - Per 128-sample block j: gathered rows land sample-on-partition in
    gb_j [128, 8, 12]; PE transposes flip them into the feature-major
    X [104, 2048]:
        rows f*12+e (e<10): emb dim e of field f
        rows f*12+10:       w_first value of field f
        rows f*12+11:       zero pad
        rows 96..102:       raw dense features (transposed on host)
        row 103:            constant 1.0 (bias row)
  - Compute in 4 column-tiles of 512 samples:
        H1  = relu(W1s^T X)           [128, 512]
        H2  = relu(W2^T H1 + b2)      [128, 512]
        SD  = sdwa^T X                [22, 512]  rows 0:10 s, 10:20 demb,
                                      20/21 = (lin+-1)/2 (difference-of-
                                      squares keeps the first-order term
                                      through the squaring stage)
        FS  = [X[0:96]^2 ; SD^2]      [118, 512]
        fin[s] = ef^T FS[:, s] + w3^T H2[:, s]   (per-128-block matmuls
                  into a PSUM tile [128, 16]; samples on partitions so
                  the sigmoid is one cheap [128,16] activation)
        out = transpose(sigmoid(fin))  -> [16, 128] -> DRAM fp32
"""

import os
from contextlib import ExitStack

import numpy as np
import ml_dtypes

import concourse.bass as bass
import concourse.bacc as bacc
import concourse.mybir as mybir
import concourse.tile as tile

# ---- problem constants (hardcoded; must match the reference) ----
VOCABS = [1000000, 500000, 200000, 100000, 50000, 10000, 5000, 1000]
S = int(np.sum(VOCABS))  # 1,866,000
OFFSETS = np.concatenate([[0], np.cumsum(VOCABS)[:-1]]).astype(np.int64)
B = 16384
EMB = 10
N_DENSE = 7
F = len(VOCABS)  # 8
HID = 128

N_CORES = 8
BL = B // N_CORES  # 2048 per core
RW = 12            # augmented table row width (10 emb + wf + pad)
KX = 104           # X partition rows: 96 gathered + 7 dense + 1 const
NSD = 22           # SD rows: 10 s + 10 demb + 2 lin-halves
NFS = 118          # FS rows: 96 emb^2 + 22 SD^2
NBLK = BL // 128   # 16 sample blocks of 128
NT = 4             # column tiles of 512

F32 = mybir.dt.float32
BF16 = mybir.dt.bfloat16
I32 = mybir.dt.int32
NPBF16 = ml_dtypes.bfloat16

# gather channel assignment: list of channel names cycled over the 128
# gather instructions.  "pool" = gpsimd SWDGE (always works); "sp"/"act" =
# HWDGE rings (only if the ucode supports indirect on them).
GATHER_CHANNELS = ["pool"]

_cached = {}


def _indirect_on(eng, queue, out, in_, off_ap):
    """indirect_dma_start with custom engine + queue (HWDGE rings)."""
    self = eng
    out_ap = self.lower_ap_dma(out, for_indirect_dma=True)
    in_ap = self.lower_ap_dma(in_, for_indirect_dma=True)
    assert len(in_ap) == 1 and len(out_ap) == 1
    offset_ap_l = self.lower_ap_dma(off_ap)[0]
    in_ap.append(offset_ap_l)
    ap_shape = in_.shape
    coef = 1
    for i in range(1, len(ap_shape)):
        coef *= ap_shape[i]
    dynamic_ap_info = mybir.DynamicAccessPatternInfo(
        c=0,
        actual_ap=out.ap,
        indirect_dim_max_index=ap_shape[0],
        offset_expr=[
            mybir.DynamicAccessPatternOffsetExpr(
                coef=coef,
                aff_expr=mybir.DynamicAccessPatternOffsetExprAffExpr(
                    kind="IndirectArgId", arg_id=1,
                ),
            )
        ],
    )
    in_ap[0].dynamic_ap_info = dynamic_ap_info
    return self.add_instruction(
        mybir.InstDMACopy(
            name=self.bass.get_next_instruction_name(),
            queue=queue,
            mode="Copy",
            ins=in_ap,
            outs=out_ap,
            oob_is_err=True,
            cce_op=mybir.AluOpType.bypass,
        )
    )


def _build_program(debug_dump=False):
    """Build the SPMD Bass program (same for all cores)."""
    nc = bacc.Bacc("TRN2", target_bir_lowering=False, debug=False)

    tab_d = nc.dram_tensor("tab", [S, RW], BF16, kind="ExternalInput").ap()
    idx_d = nc.dram_tensor("idxs", [128, 128], I32, kind="ExternalInput").ap()
    dn8_d = nc.dram_tensor("dn8", [8, BL], BF16, kind="ExternalInput").ap()
    # bf16 weights packed into one tensor: one DMA, one sem wait
    # cols: idn 0:128 | w1s 128:256 | w2 256:384 | sdwa 384:406 |
    #       ef 406 | w3 407
    wpk_d = nc.dram_tensor("wpkb", [128, 408], BF16, kind="ExternalInput").ap()
    bfp_d = nc.dram_tensor("bfp", [128, 1], F32, kind="ExternalInput").ap()
    out_d = nc.dram_tensor("out", [16, 128], F32, kind="ExternalOutput").ap()
    if debug_dump:
        xdmp_d = nc.dram_tensor("xdmp", [KX, BL], BF16,
                                kind="ExternalOutput").ap()

    with ExitStack() as ctx:
        tc = ctx.enter_context(tile.TileContext(nc))
        const = ctx.enter_context(tc.tile_pool(name="const", bufs=1))
        hpool = ctx.enter_context(tc.tile_pool(name="h", bufs=2))
        fspool = ctx.enter_context(tc.tile_pool(name="fs", bufs=2))
        pp_x = ctx.enter_context(tc.tile_pool(name="ppx", bufs=2, space="PSUM"))
        pp_h = ctx.enter_context(tc.tile_pool(name="pph", bufs=2, space="PSUM"))
        pp_s = ctx.enter_context(tc.tile_pool(name="pps", bufs=2, space="PSUM"))
        pp_f = ctx.enter_context(tc.tile_pool(name="ppf", bufs=1, space="PSUM"))

        # index tile first: the gathers depend only on it
        idx_t = const.tile([128, 128], I32)
        nc.sync.dma_start(idx_t[:], idx_d[:])

        # constants: wpk on the Act HWDGE ring so it doesn't delay the
        # gathers behind the Sync queue
        wpk_t = const.tile([128, 408], BF16)
        nc.scalar.dma_start(wpk_t[:], wpk_d[:])
        idn_t = wpk_t[:, 0:128]
        w1s_t = wpk_t[0:KX, 128:256]
        w2_t = wpk_t[:, 256:384]
        sdwa_t = wpk_t[0:KX, 384:406]
        ef_t = wpk_t[0:NFS, 406:407]
        w3_t = wpk_t[:, 407:408]
        b2_t = const.tile([128, 1], F32)
        nc.scalar.dma_start(b2_t[:], bfp_d[:])

        # X: feature-major activations
        x_t = const.tile([KX, BL], BF16)
        nc.sync.dma_start(x_t[96:104, :], dn8_d[:])

        # gathered rows, sample-on-partition, one tile per 128-sample block
        chan = {"pool": None, "sp": None, "act": None}
        gbs = []
        ci = 0
        for j in range(NBLK):
            gb = const.tile([128, F, RW], BF16, name=f"gb{j}")
            gbs.append(gb)
            for f in range(F):
                c = GATHER_CHANNELS[ci % len(GATHER_CHANNELS)]
                ci += 1
                off = bass.IndirectOffsetOnAxis(
                    ap=idx_t[:, j * F + f:j * F + f + 1], axis=0)
                if c == "pool":
                    nc.gpsimd.indirect_dma_start(
                        out=gb[:, f, :], out_offset=None,
                        in_=tab_d[:], in_offset=off)
                elif c == "sp":
                    _indirect_on(nc.sync, "qSPDynamicHW",
                                 gb[:, f, :], tab_d[:], off.ap)
                else:
                    _indirect_on(nc.scalar, "qActDynamicHW",
                                 gb[:, f, :], tab_d[:], off.ap)

        fin_sb = const.tile([128, 16], BF16)
        outT = const.tile([16, 128], F32)

        RELU = mybir.ActivationFunctionType.Relu
        SIGMOID = mybir.ActivationFunctionType.Sigmoid
        COPY = mybir.ActivationFunctionType.Copy

        finp = pp_f.tile([128, 16], F32, tag="fin")

        for t in range(NT):
            cols = slice(512 * t, 512 * (t + 1))
            # 4 PE transposes -> one PSUM tile [96, 512], one copy to SBUF
            xp = pp_x.tile([96, 512], BF16, tag="xp")
            for jj in range(4):
                j = 4 * t + jj
                nc.tensor.transpose(
                    out=xp[:, 128 * jj:128 * (jj + 1)],
                    in_=gbs[j][:],
                    identity=idn_t,
                )
            if t % 2 == 0:
                nc.vector.tensor_copy(x_t[0:96, cols], xp[:])
            else:
                nc.scalar.activation(x_t[0:96, cols], xp[:], COPY)

            # MLP
            h1p = pp_h.tile([HID, 512], F32, tag="hp")
            nc.tensor.matmul(out=h1p[:], lhsT=w1s_t, rhs=x_t[:, cols],
                             start=True, stop=True)
            h1_t = hpool.tile([HID, 512], BF16, tag="h1")
            nc.vector.tensor_scalar_max(h1_t[:], h1p[:], 0.0)
            h2p = pp_h.tile([HID, 512], F32, tag="hp")
            nc.tensor.matmul(out=h2p[:], lhsT=w2_t, rhs=h1_t[:],
                             start=True, stop=True)
            h2_t = hpool.tile([HID, 512], BF16, tag="h2")
            nc.scalar.activation(h2_t[:], h2p[:], RELU, bias=b2_t[:])

            # s / dense_emb / lin-halves rows
            sdp = pp_s.tile([NSD, 512], F32, tag="sd")
            nc.tensor.matmul(out=sdp[:], lhsT=sdwa_t, rhs=x_t[:, cols],
                             start=True, stop=True)

            # FS: squares stack
            fs_t = fspool.tile([NFS, 512], BF16, tag="fs")
            nc.vector.tensor_mul(fs_t[0:96, :], x_t[0:96, cols],
                                 x_t[0:96, cols])
            nc.scalar.activation(fs_t[96:NFS, :], sdp[:],
                                 mybir.ActivationFunctionType.Square)

            # fin accumulation: per 128-block, samples on partitions
            for jj in range(4):
                j = 4 * t + jj
                bc = slice(128 * jj, 128 * (jj + 1))
                nc.tensor.matmul(out=finp[:, j:j + 1],
                                 lhsT=fs_t[:, bc],
                                 rhs=ef_t,
                                 start=True, stop=False)
                nc.tensor.matmul(out=finp[:, j:j + 1],
                                 lhsT=h2_t[:, bc],
                                 rhs=w3_t,
                                 start=False, stop=True)

        nc.scalar.activation(fin_sb[:], finp[:], SIGMOID)
        ftp = pp_f.tile([16, 128], BF16, tag="ftp")
        nc.tensor.transpose(out=ftp[:], in_=fin_sb[:], identity=idn_t)
        nc.vector.tensor_copy(outT[:], ftp[:])
        nc.sync.dma_start(out_d[:], outT[:])
        if debug_dump:
            nc.sync.dma_start(xdmp_d[:], x_t[:])

    nc.compile()
    return nc


def _host_prep(sparse_feature, dense_feature, emb_table, W_dense, b_dense,
               w_first, b_first, W1, b1, W2, b2, W3, b3):
    """Build the augmented table, folded weights, and per-core in_maps."""
    f32 = np.float32
    emb_table = np.asarray(emb_table, dtype=f32)
    W_dense = np.asarray(W_dense, dtype=f32)      # [10, 7]
    b_dense = np.asarray(b_dense, dtype=f32)      # [10]
    w_first = np.asarray(w_first, dtype=f32)      # [S+7]
    b_first = np.asarray(b_first, dtype=f32)      # [1]
    W1 = np.asarray(W1, dtype=f32)                # [90, 128]
    b1 = np.asarray(b1, dtype=f32)                # [128]
    W2 = np.asarray(W2, dtype=f32)                # [128, 128]
    b2 = np.asarray(b2, dtype=f32)                # [128]
    W3 = np.asarray(W3, dtype=f32)                # [128, 1]
    b3 = np.asarray(b3, dtype=f32)                # [1]

    tab = np.zeros((S, RW), dtype=f32)
    tab[:, :EMB] = emb_table
    tab[:, EMB] = w_first[:S]
    tab = tab.astype(NPBF16)

    w1s = np.zeros((KX, HID), dtype=f32)
    for f in range(F):
        w1s[f * RW:f * RW + EMB] = W1[f * EMB:(f + 1) * EMB]
    w1s[96:103] = W_dense.T @ W1[F * EMB:]               # [7,128]
    w1s[103] = b1 + b_dense @ W1[F * EMB:]

    # sdwa: cols 0:10 -> s, 10:20 -> dense_emb, 20/21 -> (lin+-1)/2
    sdwa = np.zeros((KX, NSD), dtype=f32)
    for f in range(F):
        for e in range(EMB):
            sdwa[f * RW + e, e] = 1.0
    sdwa[96:103, 0:10] = W_dense.T
    sdwa[103, 0:10] = b_dense
    sdwa[96:103, 10:20] = W_dense.T
    sdwa[103, 10:20] = b_dense
    a1 = np.zeros(KX, dtype=f32)
    for f in range(F):
        a1[f * RW + EMB] = 1.0
    a1[96:103] = w_first[S:]
    a1[103] = b_first[0] + b3[0]
    sdwa[:, 20] = 0.5 * a1
    sdwa[103, 20] = 0.5 * a1[103] + 0.5
    sdwa[:, 21] = 0.5 * a1
    sdwa[103, 21] = 0.5 * a1[103] - 0.5

    # ef: weights for the FS (squares) stack
    ef = np.zeros(NFS, dtype=f32)
    for f in range(F):
        ef[f * RW:f * RW + EMB] = -0.5     # -0.5 * sum_f v^2
    ef[96:106] = 0.5                        # +0.5 * s^2
    ef[106:116] = -0.5                      # -0.5 * demb^2
    ef[116] = 1.0                           # +((lin+1)/2)^2
    ef[117] = -1.0                          # -((lin-1)/2)^2

    idx_g = (np.asarray(sparse_feature, dtype=np.int64)
             + OFFSETS[None, :]).astype(np.int32)         # [B, F]
    dense = np.asarray(dense_feature, dtype=f32)          # [B, 7]

    wpkb = np.zeros((128, 408), dtype=f32)
    wpkb[:, 0:128] = np.eye(128, dtype=f32)
    wpkb[0:KX, 128:256] = w1s
    wpkb[:, 256:384] = W2
    wpkb[0:KX, 384:406] = sdwa
    wpkb[0:NFS, 406] = ef
    wpkb[:, 407] = W3.reshape(HID)
    wpkb = wpkb.astype(NPBF16)

    bfp = b2.reshape(128, 1).astype(f32)

    common = {"tab": tab, "wpkb": wpkb, "bfp": bfp}
    in_maps = []
    for c in range(N_CORES):
        lo, hi = c * BL, (c + 1) * BL
        lg = idx_g[lo:hi].reshape(NBLK, 128, F)
        idxs = np.ascontiguousarray(
            lg.transpose(1, 0, 2).reshape(128, NBLK * F))  # [128, 128]
        dn8 = np.ones((8, BL), dtype=f32)
        dn8[:7] = dense[lo:hi].T
        in_maps.append(dict(common, idxs=idxs, dn8=dn8.astype(NPBF16)))
    return in_maps


def _get_program(debug_dump=False):
    key = ("nc", debug_dump)
    if key not in _cached:
        _cached[key] = _build_program(debug_dump)
    return _cached[key]


def run_on_device(in_maps, trace=False, debug_dump=False):
    """Run the SPMD program on 8 NeuronCores.  Returns (results, exec_time_ns)."""
    from concourse.bass_utils import run_bass_kernel_spmd

    nc = _get_program(debug_dump)
    res = run_bass_kernel_spmd(nc, in_maps, list(range(N_CORES)), trace=trace)
    return res.results, res.exec_time_ns


def kernel(**inputs):
    in_maps = _host_prep(**inputs)
    results, _ = run_on_device(in_maps, trace=False)
    out = np.concatenate(
        [results[c]["out"].reshape(BL) for c in range(N_CORES)])
    return out.astype(np.float32)


# revision 26
# speedup vs baseline: 1.0112x; 1.0112x over previous
"""DeepFM (nn_DeepFM_77558519431939) Trainium2 Bass kernel.

Strategy (8 NeuronCores, SPMD, no collectives):
  - Replicate the embedding table on every core; data-parallel the batch
    (16384 samples -> 2048 per core).  Each gathered row is fetched exactly
    once across the fleet, and there is no all-to-all.
  - Host-side prep builds an augmented table [S, 12]: 10 embedding dims,
    w_first value (first-order weight) in col 10, zero pad in col 11.  One
    indirect-DMA gather per 4096 rows fetches embeddings AND first-order
    weights together.
  - Gathered rows land sample-on-partition; PE transposes flip them into a
    feature-major activation matrix X [104, 2048]:
        rows f*12+e (e<10): emb dim e of field f
        rows f*12+10:       w_first value of field f
        rows f*12+11:       zero pad
        rows 96..102:       raw dense features (transposed on host)
        row 103:            constant 1.0 (bias row)
  - The whole DeepFM head is then a handful of matmuls per 512-column tile
    with all the small weights folded on the host:
        H1 = relu(W1s^T X)            (dense-proj + b1 folded into W1s)
        H2 = relu(W2^T H1 + b2)
        SD = SDw^T X                  (rows 0..9 = s, 10..19 = dense_emb,
                                       row 20 = first-order linear term)
        XSQ = [X[0:96]^2 ; SD[0:20]^2 ; SD[20]]
        FIN = esq^T XSQ + W3^T H2     (esq = +-0.5 masks + lin passthrough)
        out = sigmoid(FIN)
"""

import os
from contextlib import ExitStack

import numpy as np

import concourse.bass as bass
import concourse.bacc as bacc
import concourse.mybir as mybir
import concourse.tile as tile

# ---- problem constants (hardcoded; must match the reference) ----
VOCABS = [1000000, 500000, 200000, 100000, 50000, 10000, 5000, 1000]
S = int(np.sum(VOCABS))  # 1,866,000
OFFSETS = np.concatenate([[0], np.cumsum(VOCABS)[:-1]]).astype(np.int64)
B = 16384
EMB = 10
N_DENSE = 7
F = len(VOCABS)  # 8
HID = 128

N_CORES = 8
BL = B // N_CORES  # 2048 per core
RW = 12            # augmented table row width (10 emb + wf + pad)
KX = 104           # X partition rows: 96 gathered + 7 dense + 1 const
NSQ = 117          # XSQ rows: 96 emb^2 + 10 s^2 + 10 demb^2 + 1 lin
NBLK = BL // 128   # 16 sample blocks of 128
NT = BL // 512     # 4 column tiles of 512
GCH = 4            # gather chunk count (4 blocks of samples each)

F32 = mybir.dt.float32
I32 = mybir.dt.int32

_cached = {}


def _build_program(debug_dump=False):
    """Build the SPMD Bass program (same for all cores)."""
    # 64KB SWDGE descriptor ring (default 16KB): lets the Q7 run several
    # indirect-DMA instructions ahead of the SDMA drain so the gather chain
    # isn't throttled by ring credit.
    nc = bacc.Bacc("TRN2", target_bir_lowering=False, debug=False,
                   dynamic_dma_scratch_size=1 << 16)

    tab_d = nc.dram_tensor("tab", [S, RW], F32, kind="ExternalInput").ap()
    idx_d = nc.dram_tensor("idxs", [128, 128], I32, kind="ExternalInput").ap()
    dn8_d = nc.dram_tensor("dn8", [8, BL], F32, kind="ExternalInput").ap()
    # all small weights packed into one tensor: one DMA, one sem wait
    # cols: idn 0:128 | w1s 128:256 | w2 256:384 | b2 384 | sdw 385:405 |
    #       a1 405 | esq 406 | es2 407 | w3 408
    wpk_d = nc.dram_tensor("wpk", [128, 409], F32, kind="ExternalInput").ap()
    out_d = nc.dram_tensor("out", [1, BL], F32, kind="ExternalOutput").ap()
    if debug_dump:
        xdmp_d = nc.dram_tensor("xdmp", [KX, BL], F32, kind="ExternalOutput").ap()
        fdmp_d = nc.dram_tensor("fdmp", [1, BL], F32, kind="ExternalOutput").ap()

    with ExitStack() as ctx:
        tc = ctx.enter_context(tile.TileContext(nc))
        const = ctx.enter_context(tc.tile_pool(name="const", bufs=1))
        gpool = ctx.enter_context(tc.tile_pool(name="gch", bufs=128))
        hpool = ctx.enter_context(tc.tile_pool(name="h", bufs=2))
        qpool = ctx.enter_context(tc.tile_pool(name="xsq", bufs=2))
        pp_x = ctx.enter_context(tc.tile_pool(name="ppx", bufs=2, space="PSUM"))
        pp_h = ctx.enter_context(tc.tile_pool(name="pph", bufs=2, space="PSUM"))
        pp_s = ctx.enter_context(tc.tile_pool(name="pps", bufs=2, space="PSUM"))
        pp_f = ctx.enter_context(tc.tile_pool(name="ppf", bufs=2, space="PSUM"))

        # index tile first: the gathers depend only on it
        idx_t = const.tile([128, 128], I32)
        nc.sync.dma_start(idx_t[:], idx_d[:])

        # constants: one packed tile, sliced below
        wpk_t = const.tile([128, 409], F32)
        nc.sync.dma_start(wpk_t[:], wpk_d[:])
        idn_t = wpk_t[:, 0:128]
        w1s_t = wpk_t[0:KX, 128:256]
        w2_t = wpk_t[:, 256:384]
        b2_t = wpk_t[:, 384:385]
        sdw_t = wpk_t[0:KX, 385:405]
        a1_t = wpk_t[0:KX, 405:406]
        esq_t = wpk_t[0:96, 406:407]
        es2_t = wpk_t[0:20, 407:408]
        w3_t = wpk_t[:, 408:409]

        # X: feature-major activations
        x_t = const.tile([KX, BL], F32)
        nc.sync.dma_start(x_t[96:104, :], dn8_d[:])

        out_sb = const.tile([1, BL], F32)
        if debug_dump:
            fin_sb = const.tile([1, BL], F32)

        RELU = mybir.ActivationFunctionType.Relu
        SQUARE = mybir.ActivationFunctionType.Square
        SIGMOID = mybir.ActivationFunctionType.Sigmoid

        for j in range(NBLK):
            cols = slice(128 * j, 128 * (j + 1))
            gb = gpool.tile([128, F, RW], F32, tag="gch")
            for f in range(F):
                # HW indirect DMA: one offset per partition per instruction
                nc.gpsimd.indirect_dma_start(
                    out=gb[:, f, :],
                    out_offset=None,
                    in_=tab_d[:],
                    in_offset=bass.IndirectOffsetOnAxis(
                        ap=idx_t[:, j * F + f:j * F + f + 1], axis=0
                    ),
                )
            xp = pp_x.tile([96, 128], F32, tag="xp")
            nc.tensor.transpose(out=xp[:], in_=gb[:], identity=idn_t)
            nc.vector.tensor_copy(x_t[0:96, cols], xp[:])

            # MLP
            h1p = pp_h.tile([HID, 128], F32, tag="hp")
            nc.tensor.matmul(out=h1p[:], lhsT=w1s_t, rhs=x_t[:, cols],
                             start=True, stop=True)
            h1_t = hpool.tile([HID, 128], F32, tag="h")
            nc.scalar.activation(h1_t[:], h1p[:], RELU)
            h2p = pp_h.tile([HID, 128], F32, tag="hp")
            nc.tensor.matmul(out=h2p[:], lhsT=w2_t, rhs=h1_t[:],
                             start=True, stop=True)
            h2_t = hpool.tile([HID, 128], F32, tag="h")
            nc.scalar.activation(h2_t[:], h2p[:], RELU, bias=b2_t)

            # s / dense_emb rows
            sdp = pp_s.tile([20, 128], F32, tag="sd")
            nc.tensor.matmul(out=sdp[:], lhsT=sdw_t, rhs=x_t[:, cols],
                             start=True, stop=True)

            xsq = qpool.tile([96, 128], F32, tag="xsq")
            nc.vector.tensor_mul(xsq[:], x_t[0:96, cols], x_t[0:96, cols])
            sd2 = qpool.tile([20, 128], F32, tag="sd2")
            nc.scalar.activation(sd2[:], sdp[:], SQUARE)

            # final accumulation + sigmoid
            fin = pp_f.tile([1, 128], F32, tag="fin")
            nc.tensor.matmul(out=fin[:], lhsT=a1_t, rhs=x_t[:, cols],
                             start=True, stop=False)
            nc.tensor.matmul(out=fin[:], lhsT=esq_t, rhs=xsq[:],
                             start=False, stop=False)
            nc.tensor.matmul(out=fin[:], lhsT=es2_t, rhs=sd2[:],
                             start=False, stop=False)
            nc.tensor.matmul(out=fin[:], lhsT=w3_t, rhs=h2_t[:],
                             start=False, stop=True)
            if debug_dump:
                nc.vector.tensor_copy(fin_sb[:, cols], fin[:])
            nc.scalar.activation(out_sb[:, cols], fin[:], SIGMOID)

        nc.sync.dma_start(out_d[:], out_sb[:])
        if debug_dump:
            nc.sync.dma_start(xdmp_d[:], x_t[:])
            nc.sync.dma_start(fdmp_d[:], fin_sb[:])

    nc.compile()
    return nc


def _host_prep(sparse_feature, dense_feature, emb_table, W_dense, b_dense,
               w_first, b_first, W1, b1, W2, b2, W3, b3):
    """Build the augmented table, folded weights, and per-core in_maps."""
    f32 = np.float32
    emb_table = np.asarray(emb_table, dtype=f32)
    W_dense = np.asarray(W_dense, dtype=f32)      # [10, 7]
    b_dense = np.asarray(b_dense, dtype=f32)      # [10]
    w_first = np.asarray(w_first, dtype=f32)      # [S+7]
    b_first = np.asarray(b_first, dtype=f32)      # [1]
    W1 = np.asarray(W1, dtype=f32)                # [90, 128]
    b1 = np.asarray(b1, dtype=f32)                # [128]
    W2 = np.asarray(W2, dtype=f32)                # [128, 128]
    b2 = np.asarray(b2, dtype=f32)                # [128]
    W3 = np.asarray(W3, dtype=f32)                # [128, 1]
    b3 = np.asarray(b3, dtype=f32)                # [1]

    tab = np.zeros((S, RW), dtype=f32)
    tab[:, :EMB] = emb_table
    tab[:, EMB] = w_first[:S]

    w1s = np.zeros((KX, HID), dtype=f32)
    for f in range(F):
        w1s[f * RW:f * RW + EMB] = W1[f * EMB:(f + 1) * EMB]
    w1s[96:103] = W_dense.T @ W1[F * EMB:]               # [7,128]
    w1s[103] = b1 + b_dense @ W1[F * EMB:]

    sdw = np.zeros((KX, 20), dtype=f32)
    for f in range(F):
        for e in range(EMB):
            sdw[f * RW + e, e] = 1.0
    sdw[96:103, 0:10] = W_dense.T
    sdw[103, 0:10] = b_dense
    sdw[96:103, 10:20] = W_dense.T
    sdw[103, 10:20] = b_dense

    a1 = np.zeros((KX, 1), dtype=f32)
    for f in range(F):
        a1[f * RW + EMB] = 1.0
    a1[96:103, 0] = w_first[S:]
    a1[103] = b_first[0] + b3[0]

    esq = np.zeros((96, 1), dtype=f32)
    for f in range(F):
        esq[f * RW:f * RW + EMB] = -0.5
    es2 = np.zeros((20, 1), dtype=f32)
    es2[0:10] = 0.5
    es2[10:20] = -0.5

    idx_g = (np.asarray(sparse_feature, dtype=np.int64)
             + OFFSETS[None, :]).astype(np.int32)         # [B, F]
    dense = np.asarray(dense_feature, dtype=f32)          # [B, 7]

    wpk = np.zeros((128, 409), dtype=f32)
    wpk[:, 0:128] = np.eye(128, dtype=f32)
    wpk[0:KX, 128:256] = w1s
    wpk[:, 256:384] = W2
    wpk[:, 384] = b2
    wpk[0:KX, 385:405] = sdw
    wpk[0:KX, 405] = a1[:, 0]
    wpk[0:96, 406] = esq[:, 0]
    wpk[0:20, 407] = es2[:, 0]
    wpk[:, 408] = W3.reshape(HID)

    common = {"tab": tab, "wpk": wpk}
    in_maps = []
    for c in range(N_CORES):
        lo, hi = c * BL, (c + 1) * BL
        lg = idx_g[lo:hi].reshape(NBLK, 128, F)
        idxs = np.ascontiguousarray(
            lg.transpose(1, 0, 2).reshape(128, NBLK * F))  # [128, 128]
        dn8 = np.ones((8, BL), dtype=f32)
        dn8[:7] = dense[lo:hi].T
        in_maps.append(dict(common, idxs=idxs, dn8=dn8))
    return in_maps


def _get_program(debug_dump=False):
    key = ("nc", debug_dump)
    if key not in _cached:
        _cached[key] = _build_program(debug_dump)
    return _cached[key]


def run_on_device(in_maps, trace=False, debug_dump=False):
    """Run the SPMD program on 8 NeuronCores.  Returns (results, exec_time_ns)."""
    from concourse.bass_utils import run_bass_kernel_spmd

    nc = _get_program(debug_dump)
    res = run_bass_kernel_spmd(nc, in_maps, list(range(N_CORES)), trace=trace)
    return res.results, res.exec_time_ns


def kernel(**inputs):
    in_maps = _host_prep(**inputs)
    results, _ = run_on_device(in_maps, trace=False)
    out = np.concatenate([results[c]["out"].reshape(BL) for c in range(N_CORES)])
    return out.astype(np.float32)


# revision 27
# speedup vs baseline: 1.1264x; 1.1139x over previous
"""DeepFM (nn_DeepFM_77558519431939) Trainium2 Bass kernel.

Strategy (8 NeuronCores, SPMD, no collectives):
  - Replicate the embedding table on every core; data-parallel the batch
    (16384 samples -> 2048 per core).  Each gathered row is fetched exactly
    once across the fleet, and there is no all-to-all.
  - Host-side prep builds an augmented table [S, 12]: 10 embedding dims,
    w_first value (first-order weight) in col 10, zero pad in col 11.  One
    indirect-DMA gather per 4096 rows fetches embeddings AND first-order
    weights together.
  - Gathered rows land sample-on-partition; PE transposes flip them into a
    feature-major activation matrix X [104, 2048]:
        rows f*12+e (e<10): emb dim e of field f
        rows f*12+10:       w_first value of field f
        rows f*12+11:       zero pad
        rows 96..102:       raw dense features (transposed on host)
        row 103:            constant 1.0 (bias row)
  - The whole DeepFM head is then a handful of matmuls per 512-column tile
    with all the small weights folded on the host:
        H1 = relu(W1s^T X)            (dense-proj + b1 folded into W1s)
        H2 = relu(W2^T H1 + b2)
        SD = SDw^T X                  (rows 0..9 = s, 10..19 = dense_emb,
                                       row 20 = first-order linear term)
        XSQ = [X[0:96]^2 ; SD[0:20]^2 ; SD[20]]
        FIN = esq^T XSQ + W3^T H2     (esq = +-0.5 masks + lin passthrough)
        out = sigmoid(FIN)
"""

import os
from contextlib import ExitStack

import numpy as np

import concourse.bass as bass
import concourse.bacc as bacc
import concourse.mybir as mybir
import concourse.tile as tile

# ---- problem constants (hardcoded; must match the reference) ----
VOCABS = [1000000, 500000, 200000, 100000, 50000, 10000, 5000, 1000]
S = int(np.sum(VOCABS))  # 1,866,000
OFFSETS = np.concatenate([[0], np.cumsum(VOCABS)[:-1]]).astype(np.int64)
B = 16384
EMB = 10
N_DENSE = 7
F = len(VOCABS)  # 8
HID = 128

N_CORES = 8
BL = B // N_CORES  # 2048 per core
RW = 12            # augmented table row width (10 emb + wf + pad)
KX = 104           # X partition rows: 96 gathered + 7 dense + 1 const
NSQ = 117          # XSQ rows: 96 emb^2 + 10 s^2 + 10 demb^2 + 1 lin
NBLK = BL // 128   # 16 sample blocks of 128
NT = BL // 512     # 4 column tiles of 512
GCH = 4            # gather chunk count (4 blocks of samples each)

F32 = mybir.dt.float32
I32 = mybir.dt.int32

_cached = {}


def _build_program(debug_dump=False):
    """Build the SPMD Bass program (same for all cores)."""
    # 64KB SWDGE descriptor ring (default 16KB): lets the Q7 run several
    # indirect-DMA instructions ahead of the SDMA drain so the gather chain
    # isn't throttled by ring credit.
    nc = bacc.Bacc("TRN2", target_bir_lowering=False, debug=False,
                   dynamic_dma_scratch_size=1 << 16)

    tab_d = nc.dram_tensor("tab", [S, RW], F32, kind="ExternalInput").ap()
    t67_d = nc.dram_tensor("t67", [VOCABS[6] * VOCABS[7], 2 * RW], F32,
                           kind="ExternalInput").ap()
    idx_d = nc.dram_tensor("idxs", [128, 128], I32, kind="ExternalInput").ap()
    dn8_d = nc.dram_tensor("dn8", [8, BL], F32, kind="ExternalInput").ap()
    # all small weights packed into one tensor: one DMA, one sem wait
    # cols: idn 0:128 | w1s 128:256 | w2 256:384 | b2 384 | sdw 385:405 |
    #       a1 405 | esq 406 | es2 407 | w3 408
    wpk_d = nc.dram_tensor("wpk", [128, 409], F32, kind="ExternalInput").ap()
    out_d = nc.dram_tensor("out", [1, BL], F32, kind="ExternalOutput").ap()
    if debug_dump:
        xdmp_d = nc.dram_tensor("xdmp", [KX, BL], F32, kind="ExternalOutput").ap()
        fdmp_d = nc.dram_tensor("fdmp", [1, BL], F32, kind="ExternalOutput").ap()

    with ExitStack() as ctx:
        tc = ctx.enter_context(tile.TileContext(nc))
        const = ctx.enter_context(tc.tile_pool(name="const", bufs=1))
        gpool = ctx.enter_context(tc.tile_pool(name="gch", bufs=128))
        hpool = ctx.enter_context(tc.tile_pool(name="h", bufs=2))
        qpool = ctx.enter_context(tc.tile_pool(name="xsq", bufs=2))
        pp_x = ctx.enter_context(tc.tile_pool(name="ppx", bufs=2, space="PSUM"))
        pp_h = ctx.enter_context(tc.tile_pool(name="pph", bufs=2, space="PSUM"))
        pp_s = ctx.enter_context(tc.tile_pool(name="pps", bufs=2, space="PSUM"))
        pp_f = ctx.enter_context(tc.tile_pool(name="ppf", bufs=2, space="PSUM"))

        # index tile first: the gathers depend only on it
        idx_t = const.tile([128, 128], I32)
        nc.sync.dma_start(idx_t[:], idx_d[:])

        # constants: one packed tile, sliced below
        wpk_t = const.tile([128, 409], F32)
        nc.sync.dma_start(wpk_t[:], wpk_d[:])
        idn_t = wpk_t[:, 0:128]
        w1s_t = wpk_t[0:KX, 128:256]
        w2_t = wpk_t[:, 256:384]
        b2_t = wpk_t[:, 384:385]
        sdw_t = wpk_t[0:KX, 385:405]
        a1_t = wpk_t[0:KX, 405:406]
        esq_t = wpk_t[0:96, 406:407]
        es2_t = wpk_t[0:20, 407:408]
        w3_t = wpk_t[:, 408:409]

        # X: feature-major activations
        x_t = const.tile([KX, BL], F32)
        nc.sync.dma_start(x_t[96:104, :], dn8_d[:])

        out_sb = const.tile([1, BL], F32)
        if debug_dump:
            fin_sb = const.tile([1, BL], F32)

        RELU = mybir.ActivationFunctionType.Relu
        SQUARE = mybir.ActivationFunctionType.Square
        SIGMOID = mybir.ActivationFunctionType.Sigmoid

        for j in range(NBLK):
            cols = slice(128 * j, 128 * (j + 1))
            gb = gpool.tile([128, F * RW], F32, tag="gch")
            for f in range(6):
                # HW indirect DMA: one offset per partition per instruction
                nc.gpsimd.indirect_dma_start(
                    out=gb[:, f * RW:(f + 1) * RW],
                    out_offset=None,
                    in_=tab_d[:],
                    in_offset=bass.IndirectOffsetOnAxis(
                        ap=idx_t[:, j * F + f:j * F + f + 1], axis=0
                    ),
                )
            # fields 6+7 via the host-built pair table: one 96B descriptor
            # fetches both rows (row = [emb6, wf6, 0, emb7, wf7, 0])
            nc.gpsimd.indirect_dma_start(
                out=gb[:, 6 * RW:8 * RW],
                out_offset=None,
                in_=t67_d[:],
                in_offset=bass.IndirectOffsetOnAxis(
                    ap=idx_t[:, j * F + 6:j * F + 7], axis=0
                ),
            )
            xp = pp_x.tile([96, 128], F32, tag="xp")
            nc.tensor.transpose(out=xp[:], in_=gb[:], identity=idn_t)
            nc.vector.tensor_copy(x_t[0:96, cols], xp[:])

            # MLP
            h1p = pp_h.tile([HID, 128], F32, tag="hp")
            nc.tensor.matmul(out=h1p[:], lhsT=w1s_t, rhs=x_t[:, cols],
                             start=True, stop=True)
            h1_t = hpool.tile([HID, 128], F32, tag="h")
            nc.scalar.activation(h1_t[:], h1p[:], RELU)
            h2p = pp_h.tile([HID, 128], F32, tag="hp")
            nc.tensor.matmul(out=h2p[:], lhsT=w2_t, rhs=h1_t[:],
                             start=True, stop=True)
            h2_t = hpool.tile([HID, 128], F32, tag="h")
            nc.scalar.activation(h2_t[:], h2p[:], RELU, bias=b2_t)

            # s / dense_emb rows
            sdp = pp_s.tile([20, 128], F32, tag="sd")
            nc.tensor.matmul(out=sdp[:], lhsT=sdw_t, rhs=x_t[:, cols],
                             start=True, stop=True)

            xsq = qpool.tile([96, 128], F32, tag="xsq")
            nc.vector.tensor_mul(xsq[:], x_t[0:96, cols], x_t[0:96, cols])
            sd2 = qpool.tile([20, 128], F32, tag="sd2")
            nc.scalar.activation(sd2[:], sdp[:], SQUARE)

            # final accumulation + sigmoid
            fin = pp_f.tile([1, 128], F32, tag="fin")
            nc.tensor.matmul(out=fin[:], lhsT=a1_t, rhs=x_t[:, cols],
                             start=True, stop=False)
            nc.tensor.matmul(out=fin[:], lhsT=esq_t, rhs=xsq[:],
                             start=False, stop=False)
            nc.tensor.matmul(out=fin[:], lhsT=es2_t, rhs=sd2[:],
                             start=False, stop=False)
            nc.tensor.matmul(out=fin[:], lhsT=w3_t, rhs=h2_t[:],
                             start=False, stop=True)
            if debug_dump:
                nc.vector.tensor_copy(fin_sb[:, cols], fin[:])
            nc.scalar.activation(out_sb[:, cols], fin[:], SIGMOID)

        nc.sync.dma_start(out_d[:], out_sb[:])
        if debug_dump:
            nc.sync.dma_start(xdmp_d[:], x_t[:])
            nc.sync.dma_start(fdmp_d[:], fin_sb[:])

    nc.compile()
    return nc


def _host_prep(sparse_feature, dense_feature, emb_table, W_dense, b_dense,
               w_first, b_first, W1, b1, W2, b2, W3, b3):
    """Build the augmented table, folded weights, and per-core in_maps."""
    f32 = np.float32
    emb_table = np.asarray(emb_table, dtype=f32)
    W_dense = np.asarray(W_dense, dtype=f32)      # [10, 7]
    b_dense = np.asarray(b_dense, dtype=f32)      # [10]
    w_first = np.asarray(w_first, dtype=f32)      # [S+7]
    b_first = np.asarray(b_first, dtype=f32)      # [1]
    W1 = np.asarray(W1, dtype=f32)                # [90, 128]
    b1 = np.asarray(b1, dtype=f32)                # [128]
    W2 = np.asarray(W2, dtype=f32)                # [128, 128]
    b2 = np.asarray(b2, dtype=f32)                # [128]
    W3 = np.asarray(W3, dtype=f32)                # [128, 1]
    b3 = np.asarray(b3, dtype=f32)                # [1]

    tab = np.zeros((S, RW), dtype=f32)
    tab[:, :EMB] = emb_table
    tab[:, EMB] = w_first[:S]

    w1s = np.zeros((KX, HID), dtype=f32)
    for f in range(F):
        w1s[f * RW:f * RW + EMB] = W1[f * EMB:(f + 1) * EMB]
    w1s[96:103] = W_dense.T @ W1[F * EMB:]               # [7,128]
    w1s[103] = b1 + b_dense @ W1[F * EMB:]

    sdw = np.zeros((KX, 20), dtype=f32)
    for f in range(F):
        for e in range(EMB):
            sdw[f * RW + e, e] = 1.0
    sdw[96:103, 0:10] = W_dense.T
    sdw[103, 0:10] = b_dense
    sdw[96:103, 10:20] = W_dense.T
    sdw[103, 10:20] = b_dense

    a1 = np.zeros((KX, 1), dtype=f32)
    for f in range(F):
        a1[f * RW + EMB] = 1.0
    a1[96:103, 0] = w_first[S:]
    a1[103] = b_first[0] + b3[0]

    esq = np.zeros((96, 1), dtype=f32)
    for f in range(F):
        esq[f * RW:f * RW + EMB] = -0.5
    es2 = np.zeros((20, 1), dtype=f32)
    es2[0:10] = 0.5
    es2[10:20] = -0.5

    idx_g = (np.asarray(sparse_feature, dtype=np.int64)
             + OFFSETS[None, :]).astype(np.int32)         # [B, F]
    # pair table for fields 6,7: row i6*V7+i7 = [emb6,wf6,0,emb7,wf7,0]
    sp = np.asarray(sparse_feature, dtype=np.int64)
    V6, V7 = VOCABS[6], VOCABS[7]
    o6, o7 = int(OFFSETS[6]), int(OFFSETS[7])
    t67 = np.zeros((V6 * V7, 2 * RW), dtype=f32)
    t67[:, 0:EMB] = np.repeat(emb_table[o6:o6 + V6], V7, axis=0)
    t67[:, EMB] = np.repeat(w_first[o6:o6 + V6], V7)
    t67[:, RW:RW + EMB] = np.tile(emb_table[o7:o7 + V7], (V6, 1))
    t67[:, RW + EMB] = np.tile(w_first[o7:o7 + V7], V6)
    idx_g[:, 6] = (sp[:, 6] * V7 + sp[:, 7]).astype(np.int32)
    idx_g[:, 7] = 0
    dense = np.asarray(dense_feature, dtype=f32)          # [B, 7]

    wpk = np.zeros((128, 409), dtype=f32)
    wpk[:, 0:128] = np.eye(128, dtype=f32)
    wpk[0:KX, 128:256] = w1s
    wpk[:, 256:384] = W2
    wpk[:, 384] = b2
    wpk[0:KX, 385:405] = sdw
    wpk[0:KX, 405] = a1[:, 0]
    wpk[0:96, 406] = esq[:, 0]
    wpk[0:20, 407] = es2[:, 0]
    wpk[:, 408] = W3.reshape(HID)

    common = {"tab": tab, "t67": t67, "wpk": wpk}
    in_maps = []
    for c in range(N_CORES):
        lo, hi = c * BL, (c + 1) * BL
        lg = idx_g[lo:hi].reshape(NBLK, 128, F)
        idxs = np.ascontiguousarray(
            lg.transpose(1, 0, 2).reshape(128, NBLK * F))  # [128, 128]
        dn8 = np.ones((8, BL), dtype=f32)
        dn8[:7] = dense[lo:hi].T
        in_maps.append(dict(common, idxs=idxs, dn8=dn8))
    return in_maps


def _get_program(debug_dump=False):
    key = ("nc", debug_dump)
    if key not in _cached:
        _cached[key] = _build_program(debug_dump)
    return _cached[key]


def run_on_device(in_maps, trace=False, debug_dump=False):
    """Run the SPMD program on 8 NeuronCores.  Returns (results, exec_time_ns)."""
    from concourse.bass_utils import run_bass_kernel_spmd

    nc = _get_program(debug_dump)
    res = run_bass_kernel_spmd(nc, in_maps, list(range(N_CORES)), trace=trace)
    return res.results, res.exec_time_ns


def kernel(**inputs):
    in_maps = _host_prep(**inputs)
    results, _ = run_on_device(in_maps, trace=False)
    out = np.concatenate([results[c]["out"].reshape(BL) for c in range(N_CORES)])
    return out.astype(np.float32)
